# revision 1
# baseline (speedup 1.0000x reference)
"""CRF negative log-likelihood on 8 TRN2 NeuronCores.

Data-parallel over batch (128 rows/core); each core runs an identical
independent program (no collectives) and the loss only needs batch means, so
per-core partial sums are combined in numpy. Per core:

  Forward algorithm in exp space, as a 512-step PE<->DVE recurrence:
    beta' = (E^T beta) * exp(em_s - 4.5)
  with E = exp(transitions) held as a stationary bf16 matmul weight,
  augmented with a ones column at col 64 so PSUM row 64 of every product is
  the per-batch normalizer sum(beta) for free. The -4.5 exp bias keeps the
  per-step growth ~flat (host adds 512*4.5 back). The batch is split into
  NCH=2 independent chains so the PE->sem->DVE->sem->PE dependency cycle of
  one chain hides under the other's engine work (the cycle, ~585ns/step, is
  the kernel's wall-time floor; more chains raise the DVE PSUM-access cost
  faster than they hide latency).

  Rescaling (fp32 range control) every KRS steps: the normalizer row is
  captured, reciprocal'd (DVE), partition-broadcast (GpSimd), and applied
  OFF the critical path by multiplying a FUTURE step's exp(emissions) tile,
  so the recurrence never stalls; captured Z values get one batched Ln at
  the very end (exactly one extra ACT table load, keeping Exp/Copy resident
  in the activation LUT the whole run).

  Emissions are DMAed once into a [128, 64, 64]-padded resident layout;
  PE transposes fill a [128, 4, 128] PSUM tile (one full bank) and a single
  ACT exp then produces EIGHT timesteps of F at 64-aligned partitions,
  amortizing the ACT access overhead. The gold-score section is emitted
  AFTER the forward loop so the scheduler prioritizes filling the
  recurrence pipeline at startup; gold work back-fills engine gaps.

  Gold score without gathers:
   - one-hot(tags): GpSimd broadcasts int16 tags across the tag axis, DVE
     is_equal against an int16 iota runs in the 2x all-2-byte perf mode;
   - emission term: em*onehot on GpSimd, free-axis-accumulated by ACT Copy;
   - transition term: PSUM-accumulated [oh_s, oh_{s+1}] outer-product
     matmuls build a global 48x48 transition count matrix (17-step
     overlapping one-hot tiles cover the chunk-boundary pairs), read out as
     a trace against a block-diag copy of `transitions`.
"""

import numpy as np

B, S, NT = 1024, 512, 48
NCORES = 8
BL = B // NCORES  # 128 batch rows per core
CH = 16    # gold-score chunk (steps per one-hot tile)
KRS = 256   # rescale period
EMT = 64   # steps per resident emissions tile
NCH = 2    # independent forward chains (batch split)
CWS_OVERRIDE = [64,64]  # optional explicit chain widths
EXP_BIAS = 4.5  # subtracted inside exp; host adds S*EXP_BIAS back

_CACHE = {}
_LABELS = {}


def _L(instr, label):
    try:
        _LABELS[instr.ins.name] = label
    except Exception:
        pass
    return instr


def _build_nc():
    import concourse.mybir as mybir
    from concourse import bacc
    from concourse import tile

    f32 = mybir.dt.float32
    bf16 = mybir.dt.bfloat16
    i32 = mybir.dt.int32
    AF = mybir.ActivationFunctionType
    OP = mybir.AluOpType

    nc = bacc.Bacc("TRN2", target_bir_lowering=False, debug=False,
                   num_devices=NCORES)

    em_d = nc.dram_tensor("em", [BL, S, NT], f32, kind="ExternalInput")
    tg_d = nc.dram_tensor("tg", [BL, S], i32, kind="ExternalInput")
    cst_d = nc.dram_tensor("consts", [128, 418], f32, kind="ExternalInput")

    logz_d = nc.dram_tensor("logz", [1, BL], f32, kind="ExternalOutput")
    gem_d = nc.dram_tensor("goldem", [BL, 1], f32, kind="ExternalOutput")
    gtr_d = nc.dram_tensor("goldtr", [96, 1], f32, kind="ExternalOutput")

    NRS = S // KRS
    if CWS_OVERRIDE:
        cws = list(CWS_OVERRIDE)
    else:
        base = BL // NCH
        cws = [base + (1 if i < BL % NCH else 0) for i in range(NCH)]
    offs = [sum(cws[:i]) for i in range(NCH)]

    with tile.TileContext(nc) as tc:
        with (
            tc.tile_pool(name="const", bufs=1) as cpool,
            tc.tile_pool(name="emres", bufs=S // EMT) as empool,
            tc.tile_pool(name="oh", bufs=4) as ohpool,
            tc.tile_pool(name="fwd", bufs=3) as fpool,
            tc.tile_pool(name="beta", bufs=3) as bpool,
            tc.tile_pool(name="small", bufs=4) as spool,
            tc.tile_pool(name="junk", bufs=3) as jpool,
            tc.tile_pool(name="pst", bufs=2, space="PSUM") as psT,
            tc.tile_pool(name="psp", bufs=5, space="PSUM") as psP,
            tc.tile_pool(name="psc", bufs=1, space="PSUM") as psC,
        ):
            # ---- constants: one packed DMA ----
            cst = cpool.tile([128, 418], f32, tag="cst")
            nc.sync.dma_start(out=cst[:], in_=cst_d[:])
            ident = cst[:, 0:128]
            eaug_f = cst[0:NT, 128:193]
            t2 = cst[0:96, 193:289]
            eaug = cpool.tile([NT, 65], bf16, tag="eaug")
            nc.scalar.activation(eaug[:], eaug_f, AF.Copy)
            i16 = mybir.dt.int16
            iota = cpool.tile([BL, CH + 1, NT], i16, tag="iota")
            nc.gpsimd.iota(iota[:], pattern=[[0, CH + 1], [1, NT]], base=0,
                           channel_multiplier=0)
            tg = cpool.tile([BL, S], i32, tag="tg")
            nc.sync.dma_start(out=tg[:], in_=tg_d[:])
            tg16 = cpool.tile([BL, S], i16, tag="tg16")
            nc.vector.tensor_copy(tg16[:], tg[:])
            bias_ap = cpool.tile([128, 1], f32, tag="bias")
            nc.gpsimd.memset(bias_ap[:], -EXP_BIAS)

            # ---- resident emissions, padded to 64 per step ----
            emp = []
            for t in range(S // EMT):
                et = empool.tile([BL, EMT, 64], f32, tag="em")
                nc.sync.dma_start(out=et[:, :, 0:NT],
                                  in_=em_d[:, t * EMT:(t + 1) * EMT, :])
                emp.append(et)

            # ---- forward state init ----
            betas = []
            for ch in range(NCH):
                b0 = bpool.tile([NT, cws[ch]], bf16, tag=f"beta{ch}")
                nc.vector.memset(b0[:], 0.0)
                nc.vector.memset(b0[0:1, :], 1.0)
                betas.append(b0)
            # Z capture buffer: NRS rescale slots + 1 final, on partition 64
            zbuf = cpool.tile([65, (NRS + 1) * BL], f32, tag="zbuf")

            # ---- forward loop: one exp per DOUBLE step-pair (4 steps) ----
            pending = {}  # pair index -> list of (chain, psb_tile)

            def make_f4(q):
                pst = psT.tile([128, 4, BL], f32, tag="pst")
                for u in (0, 1, 2, 3):
                    p = 4 * q + u
                    te, po = divmod(p, EMT // 2)
                    _L(nc.tensor.transpose(pst[:, u, 0:64],
                                           emp[te][:, 2 * po:2 * po + 2, :],
                                        ident[:, 0:64]), "transp")
                    _L(nc.tensor.transpose(pst[:, u, 64:128],
                                           emp[te][:, 2 * po:2 * po + 2, :],
                                        ident[:, 64:128]), "transp")
                F4 = fpool.tile([128, 4, BL], bf16, tag="F2")
                _L(nc.scalar.activation(F4[:], pst[:], AF.Exp,
                                     bias=bias_ap[:, 0:1]), "exp")
                return F4

            f4_next = make_f4(0)
            for p in range(S // 2):
                q, u = divmod(p, 4)
                if u == 0:
                    F4 = f4_next
                    if 4 * q + 4 < S // 2:
                        f4_next = make_f4(q + 1)
                F2 = F4[:, u, :]
                # apply any pending rescale to this tile's EVEN step rows
                for ch, zb in pending.pop(p, []):
                    c0, cw = offs[ch], cws[ch]
                    _L(nc.vector.tensor_mul(F2[0:NT, c0:c0 + cw],
                                         F2[0:NT, c0:c0 + cw],
                                         zb[:]), "applyz")
                for sub in (0, 1):
                    s = 2 * p + sub
                    ro = 64 * sub
                    for ch in range(NCH):
                        c0, cw = offs[ch], cws[ch]
                        psp = psP.tile([65, cw], f32, tag="psp")
                        _L(nc.tensor.matmul(psp[:], eaug[:], betas[ch][:],
                                         start=True, stop=True), f"mm{ch}")
                        if s % KRS == KRS - 8:  # capture normalizer
                            r = s // KRS
                            col = r * BL + c0
                            nc.scalar.activation(
                                zbuf[64:65, col:col + cw], psp[64:65, :],
                                AF.Copy)
                            rz = spool.tile([1, cw], f32, tag="rz")
                            nc.vector.reciprocal(rz[:], psp[64:65, :])
                            zb = spool.tile([NT, cw], f32, tag="zb")
                            nc.gpsimd.partition_broadcast(zb[:], rz[:],
                                                          channels=NT)
                            pending.setdefault(p + 2, []).append((ch, zb))
                        nb = bpool.tile([NT, cws[ch]], bf16, tag=f"beta{ch}")
                        _L(nc.vector.tensor_mul(nb[:], psp[0:NT, :],
                                             F2[ro:ro + NT, c0:c0 + cw]),
                           f"fwdmul{ch}")
                        betas[ch] = nb

            # ---- gold score (independent of forward) ----
            n_chunks = S // CH
            acc_all = cpool.tile([BL, n_chunks], f32, tag="acc_all")
            cnt_mms = []
            for c in range(n_chunks):
                width = CH + 1 if c < n_chunks - 1 else CH
                oh = ohpool.tile([BL, CH + 1, NT], bf16, tag="oh")
                tgr = ohpool.tile([BL, CH + 1, NT], i16, tag="tgr")
                tgv = tg16[:, c * CH:c * CH + width, None].broadcast_to(
                    [BL, width, NT])
                _L(nc.gpsimd.tensor_copy(tgr[:, :width, :], tgv), "tgbcast")
                _L(nc.vector.tensor_tensor(oh[:, :width, :],
                                           iota[:, :width, :],
                                           tgr[:, :width, :],
                                           OP.is_equal), "cmp")
                te = c // (EMT // CH)
                so = (c % (EMT // CH)) * CH
                junk = jpool.tile([BL, CH, NT], f32, tag="junk")
                _L(nc.gpsimd.tensor_tensor(junk[:],
                                        emp[te][:, so:so + CH, 0:NT],
                                        oh[:, :CH, :], OP.mult), "goldmul")
                nc.scalar.activation(junk[:], junk[:], AF.Copy,
                                     accum_out=acc_all[:, c:c + 1])
                npairs = width - 1
                for k in range(npairs // 2):
                    cnt_mms.append((oh, 2 * k, 2, 2 * k + 1, 2))
                if npairs % 2:
                    cnt_mms.append((oh, npairs - 1, 1, npairs, 1))
            gem = cpool.tile([BL, 1], f32, tag="gem")
            nc.vector.tensor_reduce(gem[:, 0:1], acc_all[:],
                                    mybir.AxisListType.XYZW, OP.add)

            cnt = psC.tile([96, 96], f32, tag="cnt")
            for idx, (oh, l0, lw, r0, rw) in enumerate(cnt_mms):
                nc.tensor.matmul(
                    cnt[0:48 * lw, 0:48 * rw],
                    oh[:, l0:l0 + lw, :],
                    oh[:, r0:r0 + rw, :],
                    start=(idx == 0),
                    stop=(idx == len(cnt_mms) - 1),
                    skip_group_check=True,
                )
            junk2 = jpool.tile([96, 96], f32, tag="junk2")
            gtr = cpool.tile([96, 1], f32, tag="gtr")
            nc.vector.tensor_mul(junk2[:], cnt[:], t2)
            nc.vector.tensor_reduce(gtr[:, 0:1], junk2[:],
                                    mybir.AxisListType.XYZW, OP.add)
            nc.sync.dma_start(out=gtr_d[:], in_=gtr[:])
            nc.sync.dma_start(out=gem_d[:], in_=gem[:])

            # ---- final: Sigma beta, batched Ln over all Z, reduce ----
            for ch in range(NCH):
                c0, cw = offs[ch], cws[ch]
                psf = psP.tile([65, cw], f32, tag="psp")
                nc.tensor.matmul(psf[:], eaug[:], betas[ch][:], start=True,
                                 stop=True)
                nc.vector.tensor_copy(
                    zbuf[64:65, NRS * BL + c0:NRS * BL + c0 + cw],
                    psf[64:65, :])
            lnb = cpool.tile([65, (NRS + 1) * BL], f32, tag="lnb")
            nc.scalar.activation(lnb[64:65, :], zbuf[64:65, :], AF.Ln)
            red = cpool.tile([65, BL], f32, tag="red")
            v = lnb[64:65, :].rearrange("p (r b) -> p b r", b=BL)
            nc.vector.tensor_reduce(red[64:65, :], v, mybir.AxisListType.X,
                                    OP.add)
            nc.sync.dma_start(out=logz_d[:], in_=red[64:65, :])

    nc.compile()
    return nc


def _numpy_reference(emissions, transitions, tags, mask):
    em = np.transpose(emissions, (1, 0, 2)).astype(np.float64)
    tg = tags.T.astype(np.int64)
    mk = mask.T.astype(np.float64)
    seq_len, batch, num_tags = em.shape
    emit = np.take_along_axis(em, tg[..., None], axis=2)[..., 0]
    trans = transitions[tg[:-1], tg[1:]].astype(np.float64)
    score = emit[0] + (emit[1:] * mk[1:]).sum(0) + (trans * mk[1:]).sum(0)
    alphas = np.full((batch, num_tags), -10000.0)
    alphas[:, 0] = 0.0
    T64 = transitions.astype(np.float64)
    for i in range(seq_len):
        x = alphas[:, :, None] + T64[None, :, :]
        m = x.max(axis=1)
        nxt = m + np.log(np.exp(x - m[:, None, :]).sum(axis=1)) + em[i]
        mi = mk[i][:, None]
        alphas = mi * nxt + (1.0 - mi) * alphas
    m = alphas.max(axis=1)
    logZ = m + np.log(np.exp(alphas - m[:, None]).sum(axis=1))
    return np.float32((logZ - score).mean())


def kernel(emissions, transitions, tags, mask):
    emissions = np.asarray(emissions, np.float32)
    transitions = np.asarray(transitions, np.float32)
    tags = np.asarray(tags, np.int32)
    mask_arr = np.asarray(mask)
    if not np.all(mask_arr == 1):
        return _numpy_reference(emissions, transitions, tags, mask_arr)

    from concourse.bass_utils import run_bass_kernel_spmd

    if "nc" not in _CACHE:
        _CACHE["nc"] = _build_nc()
    nc = _CACHE["nc"]

    E = np.exp(transitions.astype(np.float64)).astype(np.float32)
    consts = np.zeros((128, 418), np.float32)
    consts[:, 0:128] = np.eye(128, dtype=np.float32)
    consts[0:NT, 128:176] = E
    consts[0:NT, 192] = 1.0  # eaug ones column (col 64 of the eaug view)
    consts[0:48, 193:241] = transitions
    consts[48:96, 241:289] = transitions

    in_maps = []
    for i in range(NCORES):
        sl = slice(i * BL, (i + 1) * BL)
        in_maps.append({
            "em": np.ascontiguousarray(emissions[sl]),
            "tg": np.ascontiguousarray(tags[sl]),
            "consts": consts,
        })

    _CACHE["last_in_maps"] = in_maps
    res = run_bass_kernel_spmd(nc, in_maps, core_ids=list(range(NCORES)))
    logz = np.concatenate([r["logz"][0] for r in res.results])
    logz = logz.astype(np.float64) + S * EXP_BIAS
    gold = sum(float(r["goldem"].sum()) + float(r["goldtr"].sum())
               for r in res.results)
    loss = logz.mean() - gold / B
    return np.float32(loss)



# revision 11
# speedup vs baseline: 1.5385x; 1.5385x over previous
"""CRF negative log-likelihood on 8 TRN2 NeuronCores.

Data-parallel over batch (128 rows/core); each core runs an identical
independent program (no collectives); per-core partial sums combine in numpy.

The forward algorithm (512 sequential steps in the baseline) is the wall-time
floor: each step is a PE matmul -> DVE multiply -> PE cycle of ~651ns that no
engine-level trick shortens (GpSimd cannot read PSUM; ACT scale/bias are
per-partition only). This kernel HALVES the sequential depth by splitting the
step chain in the middle:

  logZ_b = log( w^T v )   with
  v = D_{F_255} E^T ... D_{F_0} E^T e_0          (forward, steps 0..255)
  w = E D_{F_256} ... E D_{F_511} 1              (backward, steps 511..256)

Both recurrences run CONCURRENTLY as two independent 128-batch-wide chains
(fwd: beta' = (E^T beta) o F_s; bwd: w' = E (F_s o w), i.e. the same loop
shape with the transposed stationary), interleaved on PE and DVE so each
direction's handoff latency hides under the other's work. Wall time is
256 steps x ~651ns instead of 512 x 585ns.

Per direction, the stationary [48,65] matrix carries a ones column at col 64
so PSUM row 64 of every product is the per-batch normalizer for free; one
mid-run rescale per direction (captured Z, reciprocal, partition-broadcast,
applied to a FUTURE step's exp(emissions) tile off the critical path) plus a
final batched Ln keeps fp32/bf16 in range. exp(em - 4.5) keeps per-step
growth flat; the host adds 512*4.5 back.

Emissions are DMAed once into a [128, 64, 64]-padded resident layout, tiles
ordered [0,7,1,6,...] so both chain heads start immediately. PE transposes
fill a [128, 4, 128] PSUM tile and one ACT exp produces EIGHT timesteps of F
per direction per call.

Gold score (independent of the recurrences) is sprinkled into the main loop
one 16-step chunk per 8 steps so it backfills engine idle time:
 - one-hot(tags): GpSimd broadcasts int16 tags, GpSimd is_equal vs an int16
   iota (DVE is the critical engine; gold compares live on Pool);
 - emission term: em*onehot on GpSimd, free-axis-accumulated by ACT Copy;
 - transition term: PSUM-accumulated [oh_s, oh_{s+1}] outer-product matmuls
   build a 48x48 transition count matrix, read out as a trace against a
   block-diag copy of `transitions`.
"""

import numpy as np

B, S, NT = 1024, 512, 48
NCORES = 8
BL = B // NCORES  # 128 batch rows per core
CH = 16    # gold-score chunk (steps per one-hot tile)
EMT = 64   # steps per resident emissions tile
MEET = 256  # fwd handles steps [0, MEET), bwd handles [MEET, S)
RS_F = 248  # fwd rescale: capture at p=248, apply to F of step 252
RS_B = 242  # bwd rescale: capture at p=242, apply to F of step 265 (p=246)
EXP_BIAS = 4.5  # subtracted inside exp; host adds S*EXP_BIAS back

_CACHE = {}
_LABELS = {}


def _L(instr, label):
    try:
        _LABELS[instr.ins.name] = label
    except Exception:
        pass
    return instr


def _build_nc():
    import concourse.mybir as mybir
    from concourse import bacc
    from concourse import tile

    f32 = mybir.dt.float32
    bf16 = mybir.dt.bfloat16
    i32 = mybir.dt.int32
    i16 = mybir.dt.int16
    AF = mybir.ActivationFunctionType
    OP = mybir.AluOpType

    nc = bacc.Bacc("TRN2", target_bir_lowering=False, debug=False,
                   num_devices=NCORES)

    em_d = nc.dram_tensor("em", [BL, S, NT], f32, kind="ExternalInput")
    tg_d = nc.dram_tensor("tg", [BL, S], i32, kind="ExternalInput")
    cst_d = nc.dram_tensor("consts", [128, 354], f32, kind="ExternalInput")

    logz_d = nc.dram_tensor("logz", [1, BL], f32, kind="ExternalOutput")
    gem_d = nc.dram_tensor("goldem", [BL, 1], f32, kind="ExternalOutput")
    gtr_d = nc.dram_tensor("goldtr", [96, 1], f32, kind="ExternalOutput")

    NP = MEET  # loop length; fwd step s_f = p, bwd step s_b = S-1-p

    with tile.TileContext(nc) as tc:
        with (
            tc.tile_pool(name="const", bufs=1) as cpool,
            tc.tile_pool(name="emres", bufs=S // EMT) as empool,
            tc.tile_pool(name="oh", bufs=4) as ohpool,
            tc.tile_pool(name="fwdF", bufs=3) as fpF,
            tc.tile_pool(name="fwdB", bufs=3) as fpB,
            tc.tile_pool(name="beta", bufs=4) as bpool,
            tc.tile_pool(name="small", bufs=4) as spool,
            tc.tile_pool(name="junk", bufs=3) as jpool,
            tc.tile_pool(name="pst", bufs=2, space="PSUM") as psT,
            tc.tile_pool(name="pspf", bufs=2, space="PSUM") as psPF,
            tc.tile_pool(name="pspb", bufs=2, space="PSUM") as psPB,
            tc.tile_pool(name="psc", bufs=1, space="PSUM") as psC,
            tc.tile_pool(name="psz", bufs=1, space="PSUM") as psZ,
        ):
            # ---- constants: one packed DMA ----
            cst = cpool.tile([128, 354], f32, tag="cst")
            nc.sync.dma_start(out=cst[:], in_=cst_d[:])
            ident = cst[:, 0:128]
            eaugF_f = cst[0:NT, 128:193]
            eaugB_f = cst[0:NT, 193:258]
            t2 = cst[0:96, 258:354]
            eaugF = cpool.tile([NT, 65], bf16, tag="eaugF")
            nc.scalar.activation(eaugF[:], eaugF_f, AF.Copy)
            eaugB = cpool.tile([NT, 65], bf16, tag="eaugB")
            nc.scalar.activation(eaugB[:], eaugB_f, AF.Copy)
            iota = cpool.tile([BL, CH + 1, NT], i16, tag="iota")
            nc.gpsimd.iota(iota[:], pattern=[[0, CH + 1], [1, NT]], base=0,
                           channel_multiplier=0)
            tg = cpool.tile([BL, S], i32, tag="tg")
            nc.sync.dma_start(out=tg[:], in_=tg_d[:])
            tg16 = cpool.tile([BL, S], i16, tag="tg16")
            nc.vector.tensor_copy(tg16[:], tg[:])
            bias_ap = cpool.tile([128, 1], f32, tag="bias")
            nc.gpsimd.memset(bias_ap[:], -EXP_BIAS)
            ones_row = cpool.tile([1, NT], f32, tag="ones_row")
            nc.vector.memset(ones_row[:], 1.0)

            # ---- resident emissions, padded to 64 per step ----
            # order: both chain heads (tile 0 fwd, tile 7 bwd) first
            emp = [None] * (S // EMT)
            for t in (0, 7, 1, 6, 2, 5, 3, 4):
                et = empool.tile([BL, EMT, 64], f32, tag="em")
                nc.sync.dma_start(out=et[:, :, 0:NT],
                                  in_=em_d[:, t * EMT:(t + 1) * EMT, :])
                emp[t] = et

            # Z capture buffer on partition 64: [Zf | Zb | final]
            zbuf = cpool.tile([65, 3 * BL], f32, tag="zbuf")

            # ---- fwd state init: e_0 ----
            beta_f = bpool.tile([NT, BL], bf16, tag="betaf")
            nc.vector.memset(beta_f[:], 0.0)
            nc.vector.memset(beta_f[0:1, :], 1.0)

            # ---- bwd state init: F_{511} at partitions 0:48 ----
            # one standalone transpose of step S-1 so tags land at base 0
            pst_i = psT.tile([64, BL], f32, tag="pst")
            _L(nc.tensor.transpose(pst_i[:], emp[7][:, EMT - 1:EMT, :],
                                   ident[:, 0:128]), "transpI")
            beta_b = bpool.tile([NT, BL], bf16, tag="betab")
            _L(nc.scalar.activation(beta_b[:], pst_i[0:NT, :], AF.Exp,
                                    bias=bias_ap[0:NT, 0:1]), "expI")

            def make_f4(d, q):
                """8 steps of F = exp(em-4.5) for global steps 8q..8q+7,
                laid out [128, 4, 128]: partition 0:48 even step tags,
                64:112 odd step tags; batch on free."""
                pst = psT.tile([128, 4, BL], f32, tag="pst")
                for u in (0, 1, 2, 3):
                    pr = 4 * q + u
                    te, po = divmod(pr, EMT // 2)
                    _L(nc.tensor.transpose(pst[:, u, 0:64],
                                           emp[te][:, 2 * po:2 * po + 2, :],
                                           ident[:, 0:64]), "transp")
                    _L(nc.tensor.transpose(pst[:, u, 64:128],
                                           emp[te][:, 2 * po:2 * po + 2, :],
                                           ident[:, 64:128]), "transp")
                pool = fpF if d == 0 else fpB
                F4 = pool.tile([128, 4, BL], bf16, tag="F4")
                _L(nc.scalar.activation(F4[:], pst[:], AF.Exp,
                                        bias=bias_ap[:, 0:1]), "exp")
                return F4

            def fslice(F4, s):
                u = (s % 8) // 2
                ro = 64 * (s % 2)
                return F4[:, u, :], ro

            F4f_cur = make_f4(0, 0)
            F4b_cur = make_f4(1, 63)   # covers steps 504..511
            F4f_next = make_f4(0, 1)
            F4b_next = make_f4(1, 62)

            # gold bookkeeping
            n_chunks = S // CH
            acc_all = cpool.tile([BL, n_chunks], f32, tag="acc_all")
            cnt = psC.tile([96, 96], f32, tag="cnt")
            cnt_started = [False]
            pend_cnt = []   # (oh, l0, lw, r0, rw) for the previous chunk
            pend_acc = []   # (junk, c) for the previous chunk

            def emit_gold_chunk(c):
                width = CH + 1 if c < n_chunks - 1 else CH
                oh = ohpool.tile([BL, CH + 1, NT], bf16, tag="oh")
                tgr = ohpool.tile([BL, CH + 1, NT], i16, tag="tgr")
                tgv = tg16[:, c * CH:c * CH + width, None].broadcast_to(
                    [BL, width, NT])
                _L(nc.gpsimd.tensor_copy(tgr[:, :width, :], tgv), "tgbcast")
                _L(nc.vector.tensor_tensor(oh[:, :width, :],
                                           iota[:, :width, :],
                                           tgr[:, :width, :],
                                           OP.is_equal), "cmp")
                te = c // (EMT // CH)
                so = (c % (EMT // CH)) * CH
                junk = jpool.tile([BL, CH, NT], f32, tag="junk")
                _L(nc.gpsimd.tensor_tensor(junk[:],
                                           emp[te][:, so:so + CH, 0:NT],
                                           oh[:, :CH, :], OP.mult), "goldmul")
                pend_acc.append((junk, c))
                npairs = width - 1
                for k in range(npairs // 2):
                    pend_cnt.append((oh, 2 * k, 2, 2 * k + 1, 2))
                if npairs % 2:
                    pend_cnt.append((oh, npairs - 1, 1, npairs, 1))

            def flush_gold(last=False):
                for junk, c in pend_acc:
                    _L(nc.scalar.activation(junk[:], junk[:], AF.Copy,
                                            accum_out=acc_all[:, c:c + 1]),
                       "acc")
                pend_acc.clear()
                for idx, (oh, l0, lw, r0, rw) in enumerate(pend_cnt):
                    nc.tensor.matmul(
                        cnt[0:48 * lw, 0:48 * rw],
                        oh[:, l0:l0 + lw, :],
                        oh[:, r0:r0 + rw, :],
                        start=(not cnt_started[0]),
                        stop=(last and idx == len(pend_cnt) - 1),
                        skip_group_check=True,
                    )
                    cnt_started[0] = True
                pend_cnt.clear()

            # pending rescale applications: p -> list of (F4, u, ro, zbp)
            pending = {}

            def emit_capture(psp, slot, F4, p_apply, s_apply):
                """Capture Z from psp row 64, reciprocal it, broadcast it to
                48 partitions via a rank-1 PE matmul (Pool is busy with gold
                work), and divide a FUTURE step's F slice so the chain never
                stalls."""
                nc.scalar.activation(zbuf[64:65, slot * BL:(slot + 1) * BL],
                                     psp[64:65, :], AF.Copy)
                rz = spool.tile([1, BL], f32, tag="rz")
                nc.vector.reciprocal(rz[:], psp[64:65, :])
                zbp = psZ.tile([NT, BL], f32, tag="zbp")
                _L(nc.tensor.matmul(zbp[:], ones_row[:], rz[:],
                                    start=True, stop=True), "zbcast")
                u = (s_apply % 8) // 2
                ro = 64 * (s_apply % 2)
                pending.setdefault(p_apply, []).append((F4, u, ro, zbp))

            psp_b = None
            emit_gold_chunk(0)
            for p in range(NP):
                if p % 8 == 0 and p > 0:
                    q = p // 8
                    F4f_cur = F4f_next
                    F4b_cur = F4b_next
                    if q + 1 < MEET // 8:
                        F4f_next = make_f4(0, q + 1)
                    if 62 - q >= MEET // 8:
                        F4b_next = make_f4(1, 62 - q)
                    flush_gold()
                    emit_gold_chunk(q)

                for F4, u, ro, zbp in pending.pop(p, []):
                    _L(nc.vector.tensor_mul(F4[ro:ro + NT, u, :],
                                            F4[ro:ro + NT, u, :],
                                            zbp[:]), "applyz")

                # ---- fwd step s_f = p: mm then multiply ----
                psp_f = psPF.tile([65, BL], f32, tag="pspf")
                _L(nc.tensor.matmul(psp_f[:], eaugF[:], beta_f[:],
                                    start=True, stop=True), "mmF")
                if p == RS_F:
                    emit_capture(psp_f, 0, F4f_cur, p + 4, p + 4)
                F2, ro = fslice(F4f_cur, p)
                nb = bpool.tile([NT, BL], bf16, tag="betaf")
                _L(nc.vector.tensor_mul(nb[:], psp_f[0:NT, :],
                                        F2[ro:ro + NT, :]), "mulF")
                beta_f = nb

                # ---- bwd step s_b = S-1-p: multiply then mm ----
                s_b = S - 1 - p
                if p > 0:
                    F2b, rob = fslice(F4b_cur, s_b)
                    nbb = bpool.tile([NT, BL], bf16, tag="betab")
                    _L(nc.vector.tensor_mul(nbb[:], psp_b[0:NT, :],
                                            F2b[rob:rob + NT, :]), "mulB")
                    beta_b = nbb
                psp_b = psPB.tile([65, BL], f32, tag="pspb")
                _L(nc.tensor.matmul(psp_b[:], eaugB[:], beta_b[:],
                                    start=True, stop=True), "mmB")
                if p == RS_B:
                    # s_b(p=246) = 265 is in the same F4b group (264..271)
                    emit_capture(psp_b, 1, F4b_cur, p + 4, s_b - 4)

            # ---- final: c1 = v o w, Z = sum_k c1, batched Ln, reduce ----
            flush_gold(last=True)
            c1 = bpool.tile([NT, BL], bf16, tag="c1")
            _L(nc.vector.tensor_mul(c1[:], psp_b[0:NT, :], beta_f[:]),
               "combine")
            psf = psPF.tile([65, BL], f32, tag="pspf")
            nc.tensor.matmul(psf[:], eaugF[:], c1[:], start=True, stop=True)
            nc.scalar.activation(zbuf[64:65, 2 * BL:3 * BL], psf[64:65, :],
                                 AF.Copy)

            gem = cpool.tile([BL, 1], f32, tag="gem")
            nc.vector.tensor_reduce(gem[:, 0:1], acc_all[:],
                                    mybir.AxisListType.XYZW, OP.add)
            junk2 = jpool.tile([96, 96], f32, tag="junk2")
            gtr = cpool.tile([96, 1], f32, tag="gtr")
            nc.vector.tensor_mul(junk2[:], cnt[:], t2)
            nc.vector.tensor_reduce(gtr[:, 0:1], junk2[:],
                                    mybir.AxisListType.XYZW, OP.add)
            nc.sync.dma_start(out=gtr_d[:], in_=gtr[:])
            nc.sync.dma_start(out=gem_d[:], in_=gem[:])

            lnb = cpool.tile([65, 3 * BL], f32, tag="lnb")
            nc.scalar.activation(lnb[64:65, :], zbuf[64:65, :], AF.Ln)
            red = cpool.tile([65, BL], f32, tag="red")
            v = lnb[64:65, :].rearrange("p (r b) -> p b r", b=BL)
            nc.vector.tensor_reduce(red[64:65, :], v, mybir.AxisListType.X,
                                    OP.add)
            nc.sync.dma_start(out=logz_d[:], in_=red[64:65, :])

    nc.compile()
    return nc


def _numpy_reference(emissions, transitions, tags, mask):
    em = np.transpose(emissions, (1, 0, 2)).astype(np.float64)
    tg = tags.T.astype(np.int64)
    mk = mask.T.astype(np.float64)
    seq_len, batch, num_tags = em.shape
    emit = np.take_along_axis(em, tg[..., None], axis=2)[..., 0]
    trans = transitions[tg[:-1], tg[1:]].astype(np.float64)
    score = emit[0] + (emit[1:] * mk[1:]).sum(0) + (trans * mk[1:]).sum(0)
    alphas = np.full((batch, num_tags), -10000.0)
    alphas[:, 0] = 0.0
    T64 = transitions.astype(np.float64)
    for i in range(seq_len):
        x = alphas[:, :, None] + T64[None, :, :]
        m = x.max(axis=1)
        nxt = m + np.log(np.exp(x - m[:, None, :]).sum(axis=1)) + em[i]
        mi = mk[i][:, None]
        alphas = mi * nxt + (1.0 - mi) * alphas
    m = alphas.max(axis=1)
    logZ = m + np.log(np.exp(alphas - m[:, None]).sum(axis=1))
    return np.float32((logZ - score).mean())


def kernel(emissions, transitions, tags, mask):
    emissions = np.asarray(emissions, np.float32)
    transitions = np.asarray(transitions, np.float32)
    tags = np.asarray(tags, np.int32)
    mask_arr = np.asarray(mask)
    if not np.all(mask_arr == 1):
        return _numpy_reference(emissions, transitions, tags, mask_arr)

    from concourse.bass_utils import run_bass_kernel_spmd

    if "nc" not in _CACHE:
        _CACHE["nc"] = _build_nc()
    nc = _CACHE["nc"]

    E = np.exp(transitions.astype(np.float64)).astype(np.float32)
    consts = np.zeros((128, 354), np.float32)
    consts[:, 0:128] = np.eye(128, dtype=np.float32)
    consts[0:NT, 128:176] = E
    consts[0:NT, 192] = 1.0   # eaugF ones column
    consts[0:NT, 193:241] = E.T
    consts[0:NT, 257] = 1.0   # eaugB ones column
    consts[0:48, 258:306] = transitions
    consts[48:96, 306:354] = transitions

    in_maps = []
    for i in range(NCORES):
        sl = slice(i * BL, (i + 1) * BL)
        in_maps.append({
            "em": np.ascontiguousarray(emissions[sl]),
            "tg": np.ascontiguousarray(tags[sl]),
            "consts": consts,
        })

    _CACHE["last_in_maps"] = in_maps
    res = run_bass_kernel_spmd(nc, in_maps, core_ids=list(range(NCORES)))
    logz = np.concatenate([r["logz"][0] for r in res.results])
    logz = logz.astype(np.float64) + S * EXP_BIAS
    gold = sum(float(r["goldem"].sum()) + float(r["goldtr"].sum())
               for r in res.results)
    loss = logz.mean() - gold / B
    return np.float32(loss)


# revision 13
# speedup vs baseline: 1.5429x; 1.0029x over previous
"""CRF negative log-likelihood on 8 TRN2 NeuronCores.

Data-parallel over batch (128 rows/core); each core runs an identical
independent program (no collectives); per-core partial sums combine in numpy.

The forward algorithm (512 sequential steps in the baseline) is the wall-time
floor: each step is a PE matmul -> DVE multiply -> PE cycle of ~651ns that no
engine-level trick shortens (GpSimd cannot read PSUM; ACT scale/bias are
per-partition only). This kernel HALVES the sequential depth by splitting the
step chain in the middle:

  logZ_b = log( w^T v )   with
  v = D_{F_255} E^T ... D_{F_0} E^T e_0          (forward, steps 0..255)
  w = E D_{F_256} ... E D_{F_511} 1              (backward, steps 511..256)

Both recurrences run CONCURRENTLY as two independent 128-batch-wide chains
(fwd: beta' = (E^T beta) o F_s; bwd: w' = E (F_s o w), i.e. the same loop
shape with the transposed stationary), interleaved on PE and DVE so each
direction's handoff latency hides under the other's work. Wall time is
256 steps x ~651ns instead of 512 x 585ns.

Per direction, the stationary [48,65] matrix carries a ones column at col 64
so PSUM row 64 of every product is the per-batch normalizer for free; one
mid-run rescale per direction (captured Z, reciprocal, partition-broadcast,
applied to a FUTURE step's exp(emissions) tile off the critical path) plus a
final batched Ln keeps fp32/bf16 in range. exp(em - 4.5) keeps per-step
growth flat; the host adds 512*4.5 back.

Emissions are DMAed once into a [128, 64, 64]-padded resident layout, tiles
ordered [0,7,1,6,...] so both chain heads start immediately. PE transposes
fill a [128, 4, 128] PSUM tile and one ACT exp produces EIGHT timesteps of F
per direction per call.

Gold score (independent of the recurrences) is sprinkled into the main loop
one 16-step chunk per 8 steps so it backfills engine idle time:
 - one-hot(tags): GpSimd broadcasts int16 tags, GpSimd is_equal vs an int16
   iota (DVE is the critical engine; gold compares live on Pool);
 - emission term: em*onehot on GpSimd, free-axis-accumulated by ACT Copy;
 - transition term: PSUM-accumulated [oh_s, oh_{s+1}] outer-product matmuls
   build a 48x48 transition count matrix, read out as a trace against a
   block-diag copy of `transitions`.
"""

import numpy as np

B, S, NT = 1024, 512, 48
NCORES = 8
BL = B // NCORES  # 128 batch rows per core
CH = 16    # gold-score chunk (steps per one-hot tile)
EMT = 64   # steps per resident emissions tile
MEET = 256  # fwd handles steps [0, MEET), bwd handles [MEET, S)
RS_F = 248  # fwd rescale: capture at p=248, apply to F of step 252
RS_B = 242  # bwd rescale: capture at p=242, apply to F of step 265 (p=246)
EXP_BIAS = 4.5  # subtracted inside exp; host adds S*EXP_BIAS back

_CACHE = {}
_LABELS = {}


def _L(instr, label):
    try:
        _LABELS[instr.ins.name] = label
    except Exception:
        pass
    return instr


def _build_nc():
    import concourse.mybir as mybir
    from concourse import bacc
    from concourse import tile

    f32 = mybir.dt.float32
    bf16 = mybir.dt.bfloat16
    i32 = mybir.dt.int32
    i16 = mybir.dt.int16
    AF = mybir.ActivationFunctionType
    OP = mybir.AluOpType

    nc = bacc.Bacc("TRN2", target_bir_lowering=False, debug=False,
                   num_devices=NCORES)

    em_d = nc.dram_tensor("em", [BL, S, NT], f32, kind="ExternalInput")
    tg_d = nc.dram_tensor("tg", [BL, S], i32, kind="ExternalInput")
    cst_d = nc.dram_tensor("consts", [128, 354], f32, kind="ExternalInput")

    logz_d = nc.dram_tensor("logz", [1, BL], f32, kind="ExternalOutput")
    gem_d = nc.dram_tensor("goldem", [BL, 1], f32, kind="ExternalOutput")
    gtr_d = nc.dram_tensor("goldtr", [96, 1], f32, kind="ExternalOutput")

    NP = MEET  # loop length; fwd step s_f = p, bwd step s_b = S-1-p

    with tile.TileContext(nc) as tc:
        with (
            tc.tile_pool(name="const", bufs=1) as cpool,
            tc.tile_pool(name="emres", bufs=S // EMT) as empool,
            tc.tile_pool(name="oh", bufs=4) as ohpool,
            tc.tile_pool(name="fwdF", bufs=3) as fpF,
            tc.tile_pool(name="fwdB", bufs=3) as fpB,
            tc.tile_pool(name="beta", bufs=4) as bpool,
            tc.tile_pool(name="small", bufs=4) as spool,
            tc.tile_pool(name="junk", bufs=3) as jpool,
            tc.tile_pool(name="pst", bufs=3, space="PSUM") as psT,
            tc.tile_pool(name="pspf", bufs=2, space="PSUM") as psPF,
            tc.tile_pool(name="pspb", bufs=2, space="PSUM") as psPB,
            tc.tile_pool(name="psc", bufs=1, space="PSUM") as psC,
        ):
            # ---- constants: one packed DMA ----
            cst = cpool.tile([128, 354], f32, tag="cst")
            nc.sync.dma_start(out=cst[:], in_=cst_d[:])
            ident = cst[:, 0:128]
            eaugF_f = cst[0:NT, 128:193]
            eaugB_f = cst[0:NT, 193:258]
            t2 = cst[0:96, 258:354]
            eaugF = cpool.tile([NT, 65], bf16, tag="eaugF")
            nc.scalar.activation(eaugF[:], eaugF_f, AF.Copy)
            eaugB = cpool.tile([NT, 65], bf16, tag="eaugB")
            nc.scalar.activation(eaugB[:], eaugB_f, AF.Copy)
            iota = cpool.tile([BL, CH + 1, NT], i16, tag="iota")
            nc.gpsimd.iota(iota[:], pattern=[[0, CH + 1], [1, NT]], base=0,
                           channel_multiplier=0)
            tg = cpool.tile([BL, S], i32, tag="tg")
            nc.sync.dma_start(out=tg[:], in_=tg_d[:])
            tg16 = cpool.tile([BL, S], i16, tag="tg16")
            nc.vector.tensor_copy(tg16[:], tg[:])
            bias_ap = cpool.tile([128, 1], f32, tag="bias")
            nc.gpsimd.memset(bias_ap[:], -EXP_BIAS)
            ones_row = cpool.tile([1, NT], f32, tag="ones_row")
            nc.vector.memset(ones_row[:], 1.0)

            # ---- resident emissions, padded to 64 per step ----
            # order: both chain heads (tile 0 fwd, tile 7 bwd) first
            emp = [None] * (S // EMT)
            for t in (0, 7, 1, 6, 2, 5, 3, 4):
                et = empool.tile([BL, EMT, 64], f32, tag="em")
                nc.sync.dma_start(out=et[:, :, 0:NT],
                                  in_=em_d[:, t * EMT:(t + 1) * EMT, :])
                emp[t] = et

            # Z capture buffer on partition 64: [Zf | Zb | final]
            zbuf = cpool.tile([65, 3 * BL], f32, tag="zbuf")

            # ---- fwd state init: e_0 ----
            beta_f = bpool.tile([NT, BL], bf16, tag="betaf")
            nc.vector.memset(beta_f[:], 0.0)
            nc.vector.memset(beta_f[0:1, :], 1.0)

            # ---- bwd state init: F_{511} at partitions 0:48 ----
            # one standalone transpose of step S-1 so tags land at base 0
            pst_i = psT.tile([64, BL], f32, tag="pst")
            _L(nc.tensor.transpose(pst_i[:], emp[7][:, EMT - 1:EMT, :],
                                   ident[:, 0:128]), "transpI")
            beta_b = bpool.tile([NT, BL], bf16, tag="betab")
            _L(nc.scalar.activation(beta_b[:], pst_i[0:NT, :], AF.Exp,
                                    bias=bias_ap[0:NT, 0:1]), "expI")

            def make_f4(d, q):
                """8 steps of F = exp(em-4.5) for global steps 8q..8q+7,
                laid out [128, 4, 128]: partition 0:48 even step tags,
                64:112 odd step tags; batch on free."""
                pst = psT.tile([128, 4, BL], f32, tag="pst")
                for u in (0, 1, 2, 3):
                    pr = 4 * q + u
                    te, po = divmod(pr, EMT // 2)
                    _L(nc.tensor.transpose(pst[:, u, 0:64],
                                           emp[te][:, 2 * po:2 * po + 2, :],
                                           ident[:, 0:64]), "transp")
                    _L(nc.tensor.transpose(pst[:, u, 64:128],
                                           emp[te][:, 2 * po:2 * po + 2, :],
                                           ident[:, 64:128]), "transp")
                pool = fpF if d == 0 else fpB
                F4 = pool.tile([128, 4, BL], bf16, tag="F4")
                _L(nc.scalar.activation(F4[:], pst[:], AF.Exp,
                                        bias=bias_ap[:, 0:1]), "exp")
                return F4

            def fslice(F4, s):
                u = (s % 8) // 2
                ro = 64 * (s % 2)
                return F4[:, u, :], ro

            F4f_cur = make_f4(0, 0)
            F4b_cur = make_f4(1, 63)   # covers steps 504..511
            F4f_next = make_f4(0, 1)
            F4b_next = make_f4(1, 62)

            # gold bookkeeping
            n_chunks = S // CH
            acc_all = cpool.tile([BL, n_chunks], f32, tag="acc_all")
            cnt = psC.tile([96, 96], f32, tag="cnt")
            cnt_started = [False]
            pend_cnt = []   # (oh, l0, lw, r0, rw) for the previous chunk
            pend_acc = []   # (junk, c) for the previous chunk

            def emit_gold_chunk(c):
                width = CH + 1 if c < n_chunks - 1 else CH
                oh = ohpool.tile([BL, CH + 1, NT], bf16, tag="oh")
                tgr = ohpool.tile([BL, CH + 1, NT], i16, tag="tgr")
                tgv = tg16[:, c * CH:c * CH + width, None].broadcast_to(
                    [BL, width, NT])
                _L(nc.gpsimd.tensor_copy(tgr[:, :width, :], tgv), "tgbcast")
                _L(nc.vector.tensor_tensor(oh[:, :width, :],
                                           iota[:, :width, :],
                                           tgr[:, :width, :],
                                           OP.is_equal), "cmp")
                te = c // (EMT // CH)
                so = (c % (EMT // CH)) * CH
                junk = jpool.tile([BL, CH, NT], f32, tag="junk")
                _L(nc.gpsimd.tensor_tensor(junk[:],
                                           emp[te][:, so:so + CH, 0:NT],
                                           oh[:, :CH, :], OP.mult), "goldmul")
                pend_acc.append((junk, c))
                npairs = width - 1
                for k in range(npairs // 2):
                    pend_cnt.append((oh, 2 * k, 2, 2 * k + 1, 2))
                if npairs % 2:
                    pend_cnt.append((oh, npairs - 1, 1, npairs, 1))

            def flush_gold(last=False):
                for junk, c in pend_acc:
                    _L(nc.scalar.activation(junk[:], junk[:], AF.Copy,
                                            accum_out=acc_all[:, c:c + 1]),
                       "acc")
                pend_acc.clear()
                for idx, (oh, l0, lw, r0, rw) in enumerate(pend_cnt):
                    nc.tensor.matmul(
                        cnt[0:48 * lw, 0:48 * rw],
                        oh[:, l0:l0 + lw, :],
                        oh[:, r0:r0 + rw, :],
                        start=(not cnt_started[0]),
                        stop=(last and idx == len(pend_cnt) - 1),
                        skip_group_check=True,
                    )
                    cnt_started[0] = True
                pend_cnt.clear()

            # pending rescale applications: p -> list of (F4, u, ro, zbp)
            pending = {}

            def emit_capture(psp, slot, F4, p_apply, s_apply):
                """Capture Z from psp row 64, reciprocal it, broadcast it to
                48 partitions via a rank-1 PE matmul (Pool is busy with gold
                work), and divide a FUTURE step's F slice so the chain never
                stalls."""
                nc.scalar.activation(zbuf[64:65, slot * BL:(slot + 1) * BL],
                                     psp[64:65, :], AF.Copy)
                rz = spool.tile([1, BL], f32, tag="rz")
                nc.vector.reciprocal(rz[:], psp[64:65, :])
                zbp = psT.tile([NT, BL], f32, tag="pst")
                _L(nc.tensor.matmul(zbp[:], ones_row[:], rz[:],
                                    start=True, stop=True), "zbcast")
                u = (s_apply % 8) // 2
                ro = 64 * (s_apply % 2)
                pending.setdefault(p_apply, []).append((F4, u, ro, zbp))

            psp_b = None
            emit_gold_chunk(0)
            for p in range(NP):
                if p % 8 == 0 and p > 0:
                    q = p // 8
                    F4f_cur = F4f_next
                    F4b_cur = F4b_next
                    if q + 1 < MEET // 8:
                        F4f_next = make_f4(0, q + 1)
                    if 62 - q >= MEET // 8:
                        F4b_next = make_f4(1, 62 - q)
                    flush_gold()
                    emit_gold_chunk(q)

                for F4, u, ro, zbp in pending.pop(p, []):
                    _L(nc.vector.tensor_mul(F4[ro:ro + NT, u, :],
                                            F4[ro:ro + NT, u, :],
                                            zbp[:]), "applyz")

                # ---- fwd step s_f = p: mm then multiply ----
                psp_f = psPF.tile([65, BL], f32, tag="pspf")
                _L(nc.tensor.matmul(psp_f[:], eaugF[:], beta_f[:],
                                    start=True, stop=True), "mmF")
                if p == RS_F:
                    emit_capture(psp_f, 0, F4f_cur, p + 4, p + 4)
                F2, ro = fslice(F4f_cur, p)
                nb = bpool.tile([NT, BL], bf16, tag="betaf")
                _L(nc.vector.tensor_mul(nb[:], psp_f[0:NT, :],
                                        F2[ro:ro + NT, :]), "mulF")
                beta_f = nb

                # ---- bwd step s_b = S-1-p: multiply then mm ----
                s_b = S - 1 - p
                if p > 0:
                    F2b, rob = fslice(F4b_cur, s_b)
                    nbb = bpool.tile([NT, BL], bf16, tag="betab")
                    _L(nc.vector.tensor_mul(nbb[:], psp_b[0:NT, :],
                                            F2b[rob:rob + NT, :]), "mulB")
                    beta_b = nbb
                psp_b = psPB.tile([65, BL], f32, tag="pspb")
                _L(nc.tensor.matmul(psp_b[:], eaugB[:], beta_b[:],
                                    start=True, stop=True), "mmB")
                if p == RS_B:
                    # s_b(p=246) = 265 is in the same F4b group (264..271)
                    emit_capture(psp_b, 1, F4b_cur, p + 4, s_b - 4)

            # ---- final: c1 = v o w, Z = sum_k c1, batched Ln, reduce ----
            flush_gold(last=True)
            c1 = bpool.tile([NT, BL], bf16, tag="c1")
            _L(nc.vector.tensor_mul(c1[:], psp_b[0:NT, :], beta_f[:]),
               "combine")
            psf = psPF.tile([65, BL], f32, tag="pspf")
            nc.tensor.matmul(psf[:], eaugF[:], c1[:], start=True, stop=True)
            nc.scalar.activation(zbuf[64:65, 2 * BL:3 * BL], psf[64:65, :],
                                 AF.Copy)

            gem = cpool.tile([BL, 1], f32, tag="gem")
            nc.vector.tensor_reduce(gem[:, 0:1], acc_all[:],
                                    mybir.AxisListType.XYZW, OP.add)
            junk2 = jpool.tile([96, 96], f32, tag="junk2")
            gtr = cpool.tile([96, 1], f32, tag="gtr")
            nc.vector.tensor_mul(junk2[:], cnt[:], t2)
            nc.vector.tensor_reduce(gtr[:, 0:1], junk2[:],
                                    mybir.AxisListType.XYZW, OP.add)
            nc.sync.dma_start(out=gtr_d[:], in_=gtr[:])
            nc.sync.dma_start(out=gem_d[:], in_=gem[:])

            lnb = cpool.tile([65, 3 * BL], f32, tag="lnb")
            nc.scalar.activation(lnb[64:65, :], zbuf[64:65, :], AF.Ln)
            red = cpool.tile([65, BL], f32, tag="red")
            v = lnb[64:65, :].rearrange("p (r b) -> p b r", b=BL)
            nc.vector.tensor_reduce(red[64:65, :], v, mybir.AxisListType.X,
                                    OP.add)
            nc.sync.dma_start(out=logz_d[:], in_=red[64:65, :])

    nc.compile()
    return nc


def _numpy_reference(emissions, transitions, tags, mask):
    em = np.transpose(emissions, (1, 0, 2)).astype(np.float64)
    tg = tags.T.astype(np.int64)
    mk = mask.T.astype(np.float64)
    seq_len, batch, num_tags = em.shape
    emit = np.take_along_axis(em, tg[..., None], axis=2)[..., 0]
    trans = transitions[tg[:-1], tg[1:]].astype(np.float64)
    score = emit[0] + (emit[1:] * mk[1:]).sum(0) + (trans * mk[1:]).sum(0)
    alphas = np.full((batch, num_tags), -10000.0)
    alphas[:, 0] = 0.0
    T64 = transitions.astype(np.float64)
    for i in range(seq_len):
        x = alphas[:, :, None] + T64[None, :, :]
        m = x.max(axis=1)
        nxt = m + np.log(np.exp(x - m[:, None, :]).sum(axis=1)) + em[i]
        mi = mk[i][:, None]
        alphas = mi * nxt + (1.0 - mi) * alphas
    m = alphas.max(axis=1)
    logZ = m + np.log(np.exp(alphas - m[:, None]).sum(axis=1))
    return np.float32((logZ - score).mean())


def kernel(emissions, transitions, tags, mask):
    emissions = np.asarray(emissions, np.float32)
    transitions = np.asarray(transitions, np.float32)
    tags = np.asarray(tags, np.int32)
    mask_arr = np.asarray(mask)
    if not np.all(mask_arr == 1):
        return _numpy_reference(emissions, transitions, tags, mask_arr)

    from concourse.bass_utils import run_bass_kernel_spmd

    if "nc" not in _CACHE:
        _CACHE["nc"] = _build_nc()
    nc = _CACHE["nc"]

    E = np.exp(transitions.astype(np.float64)).astype(np.float32)
    consts = np.zeros((128, 354), np.float32)
    consts[:, 0:128] = np.eye(128, dtype=np.float32)
    consts[0:NT, 128:176] = E
    consts[0:NT, 192] = 1.0   # eaugF ones column
    consts[0:NT, 193:241] = E.T
    consts[0:NT, 257] = 1.0   # eaugB ones column
    consts[0:48, 258:306] = transitions
    consts[48:96, 306:354] = transitions

    in_maps = []
    for i in range(NCORES):
        sl = slice(i * BL, (i + 1) * BL)
        in_maps.append({
            "em": np.ascontiguousarray(emissions[sl]),
            "tg": np.ascontiguousarray(tags[sl]),
            "consts": consts,
        })

    _CACHE["last_in_maps"] = in_maps
    res = run_bass_kernel_spmd(nc, in_maps, core_ids=list(range(NCORES)))
    logz = np.concatenate([r["logz"][0] for r in res.results])
    logz = logz.astype(np.float64) + S * EXP_BIAS
    gold = sum(float(r["goldem"].sum()) + float(r["goldtr"].sum())
               for r in res.results)
    loss = logz.mean() - gold / B
    return np.float32(loss)


# revision 18
# speedup vs baseline: 1.5807x; 1.0245x over previous
"""CRF negative log-likelihood on 8 TRN2 NeuronCores.

Data-parallel over batch (128 rows/core); each core runs an identical
independent program (no collectives); per-core partial sums combine in numpy.

The forward algorithm (512 sequential steps in the baseline) is the wall-time
floor: each step is a PE matmul -> DVE multiply -> PE cycle of ~651ns that no
engine-level trick shortens (GpSimd cannot read PSUM; ACT scale/bias are
per-partition only). This kernel HALVES the sequential depth by splitting the
step chain in the middle:

  logZ_b = log( w^T v )   with
  v = D_{F_255} E^T ... D_{F_0} E^T e_0          (forward, steps 0..255)
  w = E D_{F_256} ... E D_{F_511} 1              (backward, steps 511..256)

Both recurrences run CONCURRENTLY as two independent 128-batch-wide chains
(fwd: beta' = (E^T beta) o F_s; bwd: w' = E (F_s o w), i.e. the same loop
shape with the transposed stationary), interleaved on PE and DVE so each
direction's handoff latency hides under the other's work. Wall time is
256 steps x ~651ns instead of 512 x 585ns.

Per direction, the stationary [48,65] matrix carries a ones column at col 64
so PSUM row 64 of every product is the per-batch normalizer for free; one
mid-run rescale per direction (captured Z, reciprocal, partition-broadcast,
applied to a FUTURE step's exp(emissions) tile off the critical path) plus a
final batched Ln keeps fp32/bf16 in range. exp(em - 4.5) keeps per-step
growth flat; the host adds 512*4.5 back.

Emissions are DMAed once into a [128, 64, 64]-padded resident layout, tiles
ordered [0,7,1,6,...] so both chain heads start immediately. PE transposes
fill a [128, 4, 128] PSUM tile and one ACT exp produces EIGHT timesteps of F
per direction per call.

Gold score (independent of the recurrences) is sprinkled into the main loop
one 16-step chunk per 8 steps so it backfills engine idle time:
 - one-hot(tags): GpSimd broadcasts int16 tags, GpSimd is_equal vs an int16
   iota (DVE is the critical engine; gold compares live on Pool);
 - emission term: em*onehot on GpSimd, free-axis-accumulated by ACT Copy;
 - transition term: PSUM-accumulated [oh_s, oh_{s+1}] outer-product matmuls
   build a 48x48 transition count matrix, read out as a trace against a
   block-diag copy of `transitions`.
"""

import numpy as np

B, S, NT = 1024, 512, 48
NCORES = 8
BL = B // NCORES  # 128 batch rows per core
CH = 16    # gold-score chunk (steps per one-hot tile)
EMT = 64   # steps per resident emissions tile
MEET = 256  # fwd handles steps [0, MEET), bwd handles [MEET, S)
RS_F = 248  # fwd rescale: capture at p=248, apply to F of step 252
RS_B = 242  # bwd rescale: capture at p=242, apply to F of step 265 (p=246)
EXP_BIAS = 4.5  # subtracted inside exp; host adds S*EXP_BIAS back

_CACHE = {}
_LABELS = {}


def _L(instr, label):
    try:
        _LABELS[instr.ins.name] = label
    except Exception:
        pass
    return instr


def _build_nc():
    import concourse.mybir as mybir
    from concourse import bacc
    from concourse import tile

    f32 = mybir.dt.float32
    bf16 = mybir.dt.bfloat16
    i32 = mybir.dt.int32
    i16 = mybir.dt.int16
    AF = mybir.ActivationFunctionType
    OP = mybir.AluOpType

    nc = bacc.Bacc("TRN2", target_bir_lowering=False, debug=False,
                   num_devices=NCORES)

    em_d = nc.dram_tensor("em", [BL, S, NT], f32, kind="ExternalInput")
    tg_d = nc.dram_tensor("tg", [BL, S], i32, kind="ExternalInput")
    cst_d = nc.dram_tensor("consts", [128, 354], f32, kind="ExternalInput")

    logz_d = nc.dram_tensor("logz", [1, BL], f32, kind="ExternalOutput")
    gem_d = nc.dram_tensor("goldem", [BL, 1], f32, kind="ExternalOutput")
    gtr_d = nc.dram_tensor("goldtr", [96, 1], f32, kind="ExternalOutput")

    NP = MEET  # loop length; fwd step s_f = p, bwd step s_b = S-1-p

    with tile.TileContext(nc) as tc:
        with (
            tc.tile_pool(name="const", bufs=1) as cpool,
            tc.tile_pool(name="emres", bufs=S // EMT) as empool,
            tc.tile_pool(name="oh", bufs=8) as ohpool,
            tc.tile_pool(name="fwdF", bufs=3) as fpF,
            tc.tile_pool(name="fwdB", bufs=3) as fpB,
            tc.tile_pool(name="beta", bufs=4) as bpool,
            tc.tile_pool(name="small", bufs=4) as spool,
            tc.tile_pool(name="junk", bufs=5) as jpool,
            tc.tile_pool(name="pst", bufs=3, space="PSUM") as psT,
            tc.tile_pool(name="pspf", bufs=2, space="PSUM") as psPF,
            tc.tile_pool(name="pspb", bufs=2, space="PSUM") as psPB,
            tc.tile_pool(name="psc", bufs=1, space="PSUM") as psC,
        ):
            # ---- constants: one packed DMA ----
            cst = cpool.tile([128, 354], f32, tag="cst")
            nc.sync.dma_start(out=cst[:], in_=cst_d[:])
            ident = cst[:, 0:128]
            eaugF_f = cst[0:NT, 128:193]
            eaugB_f = cst[0:NT, 193:258]
            t2 = cst[0:96, 258:354]
            eaugF = cpool.tile([NT, 65], bf16, tag="eaugF")
            nc.scalar.activation(eaugF[:], eaugF_f, AF.Copy)
            eaugB = cpool.tile([NT, 65], bf16, tag="eaugB")
            nc.scalar.activation(eaugB[:], eaugB_f, AF.Copy)
            iota = cpool.tile([BL, CH + 1, NT], i16, tag="iota")
            nc.gpsimd.iota(iota[:], pattern=[[0, CH + 1], [1, NT]], base=0,
                           channel_multiplier=0)
            tg = cpool.tile([BL, S], i32, tag="tg")
            nc.sync.dma_start(out=tg[:], in_=tg_d[:])
            tg16 = cpool.tile([BL, S], i16, tag="tg16")
            nc.vector.tensor_copy(tg16[:], tg[:])
            bias_ap = cpool.tile([128, 1], f32, tag="bias")
            nc.gpsimd.memset(bias_ap[:], -EXP_BIAS)
            ones_row = cpool.tile([1, NT], f32, tag="ones_row")
            nc.vector.memset(ones_row[:], 1.0)

            # ---- resident emissions, padded to 64 per step ----
            # order: both chain heads (tile 0 fwd, tile 7 bwd) first
            emp = [None] * (S // EMT)
            for t in (0, 7, 1, 6, 2, 5, 3, 4):
                et = empool.tile([BL, EMT, 64], f32, tag="em")
                nc.sync.dma_start(out=et[:, :, 0:NT],
                                  in_=em_d[:, t * EMT:(t + 1) * EMT, :])
                emp[t] = et

            # Z capture buffer on partition 64: [Zf | Zb | final]
            zbuf = cpool.tile([65, 3 * BL], f32, tag="zbuf")

            # ---- fwd state init: e_0 ----
            beta_f = bpool.tile([NT, BL], bf16, tag="betaf")
            nc.vector.memset(beta_f[:], 0.0)
            nc.vector.memset(beta_f[0:1, :], 1.0)

            # ---- bwd state init: F_{511} at partitions 0:48 ----
            # one standalone transpose of step S-1 so tags land at base 0
            pst_i = psT.tile([64, BL], f32, tag="pst")
            _L(nc.tensor.transpose(pst_i[:], emp[7][:, EMT - 1:EMT, :],
                                   ident[:, 0:128]), "transpI")
            beta_b = bpool.tile([NT, BL], bf16, tag="betab")
            _L(nc.scalar.activation(beta_b[:], pst_i[0:NT, :], AF.Exp,
                                    bias=bias_ap[0:NT, 0:1]), "expI")

            def make_f4(d, q):
                """8 steps of F = exp(em-4.5) for global steps 8q..8q+7,
                laid out [128, 4, 128]: partition 0:48 even step tags,
                64:112 odd step tags; batch on free."""
                pst = psT.tile([128, 4, BL], f32, tag="pst")
                for u in (0, 1, 2, 3):
                    pr = 4 * q + u
                    te, po = divmod(pr, EMT // 2)
                    _L(nc.tensor.transpose(pst[:, u, 0:64],
                                           emp[te][:, 2 * po:2 * po + 2, :],
                                           ident[:, 0:64]), "transp")
                    _L(nc.tensor.transpose(pst[:, u, 64:128],
                                           emp[te][:, 2 * po:2 * po + 2, :],
                                           ident[:, 64:128]), "transp")
                pool = fpF if d == 0 else fpB
                F4 = pool.tile([128, 4, BL], bf16, tag="F4")
                _L(nc.scalar.activation(F4[:], pst[:], AF.Exp,
                                        bias=bias_ap[:, 0:1]), "exp")
                return F4

            def fslice(F4, s):
                u = (s % 8) // 2
                ro = 64 * (s % 2)
                return F4[:, u, :], ro

            F4f_cur = make_f4(0, 0)
            F4b_cur = make_f4(1, 63)   # covers steps 504..511
            F4f_next = make_f4(0, 1)
            F4b_next = make_f4(1, 62)

            # gold bookkeeping
            n_chunks = S // CH
            acc_all = cpool.tile([BL, n_chunks], f32, tag="acc_all")
            cnt = psC.tile([96, 96], f32, tag="cnt")
            cnt_started = [0]
            pend_cnt = []   # (oh, l0, lw, r0, rw) for the previous chunk
            pend_acc = []   # (junk, c) for the previous chunk

            def emit_gold_chunk(c):
                width = CH + 1 if c < n_chunks - 1 else CH
                oh = ohpool.tile([BL, CH + 1, NT], bf16, tag="oh")
                tgr = ohpool.tile([BL, CH + 1, NT], i16, tag="tgr")
                tgv = tg16[:, c * CH:c * CH + width, None].broadcast_to(
                    [BL, width, NT])
                _L(nc.gpsimd.tensor_copy(tgr[:, :width, :], tgv), "tgbcast")
                _L(nc.vector.tensor_tensor(oh[:, :width, :],
                                           iota[:, :width, :],
                                           tgr[:, :width, :],
                                           OP.is_equal), "cmp")
                te = c // (EMT // CH)
                so = (c % (EMT // CH)) * CH
                junk = jpool.tile([BL, CH, NT], f32, tag="junk")
                _L(nc.gpsimd.tensor_tensor(junk[:],
                                           emp[te][:, so:so + CH, 0:NT],
                                           oh[:, :CH, :], OP.mult), "goldmul")
                pend_acc.append((junk, c))
                npairs = width - 1
                for k in range(npairs // 2):
                    pend_cnt.append((oh, 2 * k, 2, 2 * k + 1, 2))
                if npairs % 2:
                    pend_cnt.append((oh, npairs - 1, 1, npairs, 1))

            n_cnt_total = 8 * (S // CH)  # 8 pair-matmuls per chunk

            def flush_gold():
                for junk, c in pend_acc:
                    _L(nc.scalar.activation(junk[:], junk[:], AF.Copy,
                                            accum_out=acc_all[:, c:c + 1]),
                       "acc")
                pend_acc.clear()
                for oh, l0, lw, r0, rw in pend_cnt:
                    cnt_started[0] += 1
                    nc.tensor.matmul(
                        cnt[0:48 * lw, 0:48 * rw],
                        oh[:, l0:l0 + lw, :],
                        oh[:, r0:r0 + rw, :],
                        start=(cnt_started[0] == 1),
                        stop=(cnt_started[0] == n_cnt_total),
                        skip_group_check=True,
                    )
                pend_cnt.clear()

            # pending rescale applications: p -> list of (F4, u, ro, zbp)
            pending = {}

            def emit_capture(psp, slot, F4, p_apply, s_apply):
                """Capture Z from psp row 64, reciprocal it, broadcast it to
                48 partitions via a rank-1 PE matmul (Pool is busy with gold
                work), and divide a FUTURE step's F slice so the chain never
                stalls."""
                nc.scalar.activation(zbuf[64:65, slot * BL:(slot + 1) * BL],
                                     psp[64:65, :], AF.Copy)
                rz = spool.tile([1, BL], f32, tag="rz")
                nc.vector.reciprocal(rz[:], psp[64:65, :])
                zbp = psT.tile([NT, BL], f32, tag="pst")
                _L(nc.tensor.matmul(zbp[:], ones_row[:], rz[:],
                                    start=True, stop=True), "zbcast")
                u = (s_apply % 8) // 2
                ro = 64 * (s_apply % 2)
                pending.setdefault(p_apply, []).append((F4, u, ro, zbp))

            psp_b = None
            # gold chunks emitted 3 windows ahead of their flush so the
            # Pool->DVE->Pool chunk pipeline never gates the recurrence
            emit_gold_chunk(0)
            emit_gold_chunk(1)
            emit_gold_chunk(2)
            for p in range(NP):
                if p % 8 == 0 and p > 0:
                    q = p // 8
                    F4f_cur = F4f_next
                    F4b_cur = F4b_next
                    if q + 1 < MEET // 8:
                        F4f_next = make_f4(0, q + 1)
                    if 62 - q >= MEET // 8:
                        F4b_next = make_f4(1, 62 - q)
                    flush_gold()
                    if q + 2 < n_chunks:
                        emit_gold_chunk(q + 2)

                for F4, u, ro, zbp in pending.pop(p, []):
                    _L(nc.vector.tensor_mul(F4[ro:ro + NT, u, :],
                                            F4[ro:ro + NT, u, :],
                                            zbp[:]), "applyz")

                # ---- fwd step s_f = p: mm then multiply ----
                psp_f = psPF.tile([65, BL], f32, tag="pspf")
                _L(nc.tensor.matmul(psp_f[:], eaugF[:], beta_f[:],
                                    start=True, stop=True), "mmF")
                if p == RS_F:
                    emit_capture(psp_f, 0, F4f_cur, p + 4, p + 4)
                F2, ro = fslice(F4f_cur, p)
                nb = bpool.tile([NT, BL], bf16, tag="betaf")
                _L(nc.vector.tensor_mul(nb[:], psp_f[0:NT, :],
                                        F2[ro:ro + NT, :]), "mulF")
                beta_f = nb

                # ---- bwd step s_b = S-1-p: multiply then mm ----
                s_b = S - 1 - p
                if p > 0:
                    F2b, rob = fslice(F4b_cur, s_b)
                    nbb = bpool.tile([NT, BL], bf16, tag="betab")
                    _L(nc.vector.tensor_mul(nbb[:], psp_b[0:NT, :],
                                            F2b[rob:rob + NT, :]), "mulB")
                    beta_b = nbb
                psp_b = psPB.tile([65, BL], f32, tag="pspb")
                _L(nc.tensor.matmul(psp_b[:], eaugB[:], beta_b[:],
                                    start=True, stop=True), "mmB")
                if p == RS_B:
                    # s_b(p=246) = 265 is in the same F4b group (264..271)
                    emit_capture(psp_b, 1, F4b_cur, p + 4, s_b - 4)

            # ---- final: c1 = v o w, Z = sum_k c1, batched Ln, reduce ----
            flush_gold()
            c1 = bpool.tile([NT, BL], bf16, tag="c1")
            _L(nc.vector.tensor_mul(c1[:], psp_b[0:NT, :], beta_f[:]),
               "combine")
            psf = psPF.tile([65, BL], f32, tag="pspf")
            nc.tensor.matmul(psf[:], eaugF[:], c1[:], start=True, stop=True)
            nc.scalar.activation(zbuf[64:65, 2 * BL:3 * BL], psf[64:65, :],
                                 AF.Copy)

            gem = cpool.tile([BL, 1], f32, tag="gem")
            nc.vector.tensor_reduce(gem[:, 0:1], acc_all[:],
                                    mybir.AxisListType.XYZW, OP.add)
            junk2 = jpool.tile([96, 96], f32, tag="junk2")
            gtr = cpool.tile([96, 1], f32, tag="gtr")
            nc.vector.tensor_mul(junk2[:], cnt[:], t2)
            nc.vector.tensor_reduce(gtr[:, 0:1], junk2[:],
                                    mybir.AxisListType.XYZW, OP.add)
            nc.sync.dma_start(out=gtr_d[:], in_=gtr[:])
            nc.sync.dma_start(out=gem_d[:], in_=gem[:])

            lnb = cpool.tile([65, 3 * BL], f32, tag="lnb")
            nc.scalar.activation(lnb[64:65, :], zbuf[64:65, :], AF.Ln)
            red = cpool.tile([65, BL], f32, tag="red")
            v = lnb[64:65, :].rearrange("p (r b) -> p b r", b=BL)
            nc.vector.tensor_reduce(red[64:65, :], v, mybir.AxisListType.X,
                                    OP.add)
            nc.sync.dma_start(out=logz_d[:], in_=red[64:65, :])

    nc.compile()
    return nc


def _numpy_reference(emissions, transitions, tags, mask):
    em = np.transpose(emissions, (1, 0, 2)).astype(np.float64)
    tg = tags.T.astype(np.int64)
    mk = mask.T.astype(np.float64)
    seq_len, batch, num_tags = em.shape
    emit = np.take_along_axis(em, tg[..., None], axis=2)[..., 0]
    trans = transitions[tg[:-1], tg[1:]].astype(np.float64)
    score = emit[0] + (emit[1:] * mk[1:]).sum(0) + (trans * mk[1:]).sum(0)
    alphas = np.full((batch, num_tags), -10000.0)
    alphas[:, 0] = 0.0
    T64 = transitions.astype(np.float64)
    for i in range(seq_len):
        x = alphas[:, :, None] + T64[None, :, :]
        m = x.max(axis=1)
        nxt = m + np.log(np.exp(x - m[:, None, :]).sum(axis=1)) + em[i]
        mi = mk[i][:, None]
        alphas = mi * nxt + (1.0 - mi) * alphas
    m = alphas.max(axis=1)
    logZ = m + np.log(np.exp(alphas - m[:, None]).sum(axis=1))
    return np.float32((logZ - score).mean())


def kernel(emissions, transitions, tags, mask):
    emissions = np.asarray(emissions, np.float32)
    transitions = np.asarray(transitions, np.float32)
    tags = np.asarray(tags, np.int32)
    mask_arr = np.asarray(mask)
    if not np.all(mask_arr == 1):
        return _numpy_reference(emissions, transitions, tags, mask_arr)

    from concourse.bass_utils import run_bass_kernel_spmd

    if "nc" not in _CACHE:
        _CACHE["nc"] = _build_nc()
    nc = _CACHE["nc"]

    E = np.exp(transitions.astype(np.float64)).astype(np.float32)
    consts = np.zeros((128, 354), np.float32)
    consts[:, 0:128] = np.eye(128, dtype=np.float32)
    consts[0:NT, 128:176] = E
    consts[0:NT, 192] = 1.0   # eaugF ones column
    consts[0:NT, 193:241] = E.T
    consts[0:NT, 257] = 1.0   # eaugB ones column
    consts[0:48, 258:306] = transitions
    consts[48:96, 306:354] = transitions

    in_maps = []
    for i in range(NCORES):
        sl = slice(i * BL, (i + 1) * BL)
        in_maps.append({
            "em": np.ascontiguousarray(emissions[sl]),
            "tg": np.ascontiguousarray(tags[sl]),
            "consts": consts,
        })

    _CACHE["last_in_maps"] = in_maps
    res = run_bass_kernel_spmd(nc, in_maps, core_ids=list(range(NCORES)))
    logz = np.concatenate([r["logz"][0] for r in res.results])
    logz = logz.astype(np.float64) + S * EXP_BIAS
    gold = sum(float(r["goldem"].sum()) + float(r["goldtr"].sum())
               for r in res.results)
    loss = logz.mean() - gold / B
    return np.float32(loss)
